# revision 16
# baseline (speedup 1.0000x reference)
"""MinGRU Trainium2 kernel.

Reference computation (per batch b):
    c = depthwise_conv1d(x, conv_w, taps=5, pad=2)        # [D, L]
    h = h_w @ c                                           # [O, L]
    g = concat([-1000, +1000], g_w @ c)                   # [O, L]
    a = sigmoid(-g); v = (1-a) * h
    out[l] = a[l] * out[l-1] + v[l]     (linear scan along L)

Strategy: pure data-parallel over B (8 batches -> 8 NeuronCores).
Per core, everything streams in l-chunks of 512:
  - all matmul inputs bf16 (x, conv diag weights, h/g weights); PSUM f32
  - x is zero-padded to [D, L+4] on the host, so every chunk (edges
    included) is one single halo-covering DMA
  - conv: taps {0,1,3,4} as diagonal-matmuls on TensorE accumulating in
    PSUM; the center tap is fused into the PSUM->SBUF move as one DVE
    scalar_tensor_tensor (c = x2*w2 + cpsum), which saves a matmul per
    d-tile AND replaces the separate copy pass.  NDT_ACT selects how many
    d-tiles instead keep 5 PE taps with a ScalarE copy (engine rebalance
    knob; 0 = everything on the DVE merge path).
  - weights are uploaded pre-shuffled to p-major [128, ...] layouts so every
    weight DMA is one trigger with large contiguous per-partition rows
  - a = sigmoid(-(g+bias)) on ScalarE; bias carries the +/-1000 polarized
    rows, so scan rows 0/1 equal 0 / h automatically (within error budget)
  - h weights negated on the host so v = (1-a)*h = (a-1)*(-h) is one fused
    DVE scalar_tensor_tensor reading -h straight from PSUM
  - scan via tensor_tensor_scan (DVE), bf16 in/out with f32 internal state
  - dummy matmuls chained on the tail tiles keep the Tensor engine "active"
    through the drain phase so the HAM does not duty-throttle the tail
  - out stored bf16 (one combined DMA per chunk), converted to f32 on host
"""

import numpy as np
import ml_dtypes

import concourse.bass as bass
import concourse.mybir as mybir
from concourse import bacc
from concourse.tile import TileContext
from concourse.bass_utils import run_bass_kernel_spmd

F32 = mybir.dt.float32
BF16 = mybir.dt.bfloat16
AF = mybir.ActivationFunctionType
OP = mybir.AluOpType

B, D, O, L = 8, 512, 512, 4096
P = 128
CH = 512                 # l-chunk width (one PSUM bank)
CHW = CH + 4             # x chunk width incl. 2-col halo each side
NCH = L // CH            # 8
NDT = D // P             # 4 d-tiles
NOT = O // P             # 4 o-tiles
NTAPS = 5
NDT_ACT = 0              # d-tiles whose c-copy runs on ScalarE (5 PE taps)
N_CORES = 8


def build_program():
    nc = bacc.Bacc()

    xp = nc.declare_dram_parameter("xp", [D, L + 4], BF16, isOutput=False)
    # p-major pre-shuffled weight layouts (see prepare_in_maps)
    hw4d = nc.declare_dram_parameter("hw4d", [P, NDT * O], BF16, isOutput=False)
    gw4d = nc.declare_dram_parameter("gw4d", [P, NDT * O], BF16, isOutput=False)
    cw4d = nc.declare_dram_parameter("cw4d", [P, NDT * NTAPS * P], BF16,
                                     isOutput=False)
    cwm4d = nc.declare_dram_parameter("cwm4d", [P, NDT], F32, isOutput=False)
    gbn4d = nc.declare_dram_parameter("gbn4d", [P, NOT], F32, isOutput=False)
    zpad = nc.declare_dram_parameter("zpad", [P, 2], BF16, isOutput=False)
    out = nc.declare_dram_parameter("out", [O, L], BF16, isOutput=True)
    dbg = nc.declare_dram_parameter("dbg", [2, 2], F32, isOutput=True)

    CWB = NTAPS * P          # cw block stride per d-tile

    with TileContext(nc) as tc:
        with (
            tc.tile_pool(name="weights", bufs=1) as wpool,
            tc.tile_pool(name="xin", bufs=4) as xpool,
            tc.tile_pool(name="csb", bufs=10) as cpool,
            tc.tile_pool(name="actout", bufs=12) as apool,
            tc.tile_pool(name="vtiles", bufs=12) as vpool,
            tc.tile_pool(name="outt", bufs=3) as opool,
            tc.tile_pool(name="cps", bufs=3, space="PSUM") as cps_pool,
            tc.tile_pool(name="hps", bufs=3, space="PSUM") as hps_pool,
            tc.tile_pool(name="gps", bufs=2, space="PSUM") as gps_pool,
        ):
            # The warm-up tile comes from a GpSimd memset (no DMA), so the PE
            # can start its HAM-tripping dummy matmuls as soon as the initial
            # barrier clears instead of waiting ~10us for the DMA rings.
            # Scalar queue: conv weights (d-tiles 0,1 first so chunk-0 conv
            # starts early), then x chunks 2..7. Sync queue: x chunks 0,1
            # then the stores. GpSimd SWDGE: the two big h/g matrices.
            warm_sb = wpool.tile([P, 2], BF16, tag="warm")
            nc.gpsimd.memset(warm_sb, 0.0)
            if NDT_ACT > 0:
                cw4a = wpool.tile([P, NDT_ACT * CWB], BF16, tag="cw4a")
                nc.scalar.dma_start(out=cw4a, in_=cw4d[:, 0:NDT_ACT * CWB])
            cw4b = wpool.tile([P, (NDT - NDT_ACT) * CWB], BF16, tag="cw4b")
            nc.scalar.dma_start(out=cw4b, in_=cw4d[:, NDT_ACT * CWB:])
            cwm4 = wpool.tile([P, NDT], F32, tag="cwm4")
            nc.scalar.dma_start(out=cwm4, in_=cwm4d[:, :])
            gbn4 = wpool.tile([P, NOT], F32, tag="gbn4")
            nc.scalar.dma_start(out=gbn4, in_=gbn4d[:, :])
            gw4 = wpool.tile([P, NDT * O], BF16, tag="gw4")
            nc.gpsimd.dma_start(out=gw4, in_=gw4d[:, :])
            hw4 = wpool.tile([P, NDT * O], BF16, tag="hw4")
            nc.gpsimd.dma_start(out=hw4, in_=hw4d[:, :])

            def cwblk(dt, k):
                if dt < NDT_ACT:
                    return cw4a[:, dt * CWB + k * P:dt * CWB + (k + 1) * P]
                dtb = dt - NDT_ACT
                return cw4b[:, dtb * CWB + k * P:dtb * CWB + (k + 1) * P]

            c_sb = [None] * NCH          # [chunk] -> list of 4 SBUF c tiles
            prev_out = [None] * NCH      # [chunk] -> combined out tile
            tail_tiles = []              # late tiles for the HAM keep-alive

            def emit_conv(i):
                lo = i * CH
                # one combined x DMA for all 4 d-tiles (halo included; x is
                # pre-padded on the host so edge chunks need no special case).
                # The first two chunks load the d-tile-0 slice as its own
                # tile+DMA so chunk-0 conv can start as soon as it lands.
                if i < 2:
                    xta = xpool.tile([P, CHW], BF16, tag="xta")
                    nc.sync.dma_start(
                        out=xta,
                        in_=xp[0:P, lo:lo + CHW])
                    xt = xpool.tile([P, (NDT - 1) * CHW], BF16, tag="xtb")
                    nc.sync.dma_start(
                        out=xt.rearrange("p (q c) -> p q c", c=CHW),
                        in_=xp[P:, lo:lo + CHW].rearrange(
                            "(q p) l -> p q l", p=P))

                    def xs(dt, k):
                        if dt == 0:
                            return xta[:, k:k + CH]
                        return xt[:, (dt - 1) * CHW + k:(dt - 1) * CHW + k + CH]
                else:
                    xt = xpool.tile([P, NDT * CHW], BF16, tag="xt")
                    nc.scalar.dma_start(
                        out=xt.rearrange("p (q c) -> p q c", c=CHW),
                        in_=xp[:, lo:lo + CHW].rearrange(
                            "(q p) l -> p q l", p=P))

                    def xs(dt, k):
                        return xt[:, dt * CHW + k:dt * CHW + k + CH]
                tiles = []
                for dt in range(NDT):
                    on_act = dt < NDT_ACT
                    cp = cps_pool.tile([P, CH], F32, tag="cps")
                    taps = (1, 3, 0, 2, 4) if on_act else (1, 3, 0, 4)
                    for j, k in enumerate(taps):
                        nc.tensor.matmul(
                            cp,
                            lhsT=cwblk(dt, k),
                            rhs=xs(dt, k),
                            start=(j == 0), stop=(j == len(taps) - 1),
                        )
                    ct = cpool.tile([P, CH], BF16, tag="ct")
                    if on_act:
                        nc.scalar.copy(ct, cp)
                    else:
                        # center tap fused with the PSUM->SBUF move
                        nc.vector.scalar_tensor_tensor(
                            ct, xs(dt, 2), cwm4[:, dt:dt + 1], cp,
                            OP.mult, OP.add)
                    tiles.append(ct)
                c_sb[i] = tiles

            def emit_rest(i):
                lo = i * CH
                ott = opool.tile([P, NOT * CH], BF16, tag="ott")
                for ot in range(NOT):
                    # g before h: the sigmoid chain (ACT) only needs g, so it
                    # starts while the h matmuls are still streaming
                    gp = gps_pool.tile([P, CH], F32, tag="gps")
                    for dt in range(NDT):
                        nc.tensor.matmul(
                            gp,
                            lhsT=gw4[:, dt * O + ot * P:dt * O + (ot + 1) * P],
                            rhs=c_sb[i][dt],
                            start=(dt == 0), stop=(dt == NDT - 1),
                        )
                    hp = hps_pool.tile([P, CH], F32, tag="hps")
                    for dt in range(NDT):
                        nc.tensor.matmul(
                            hp,
                            lhsT=hw4[:, dt * O + ot * P:dt * O + (ot + 1) * P],
                            rhs=c_sb[i][dt],
                            start=(dt == 0), stop=(dt == NDT - 1),
                        )
                    # a = sigmoid(-(g + bias)); v = (a-1)*(-h)
                    at = apool.tile([P, CH], BF16, tag="at")
                    nc.scalar.activation(at, gp, AF.Sigmoid,
                                         bias=gbn4[:, ot:ot + 1], scale=-1.0)
                    vt = vpool.tile([P, CH], BF16, tag="vt")
                    nc.vector.scalar_tensor_tensor(vt, at, 1.0, hp,
                                                   OP.subtract, OP.mult)
                    init = (0.0 if i == 0 else
                            prev_out[i - 1][:, ot * CH + CH - 1:ot * CH + CH])
                    nc.vector.tensor_tensor_scan(
                        ott[:, ot * CH:(ot + 1) * CH], at, vt, init,
                        OP.mult, OP.add)
                    if i >= NCH - 2:
                        tail_tiles.append((at, vt))
                nc.sync.dma_start(
                    out=out[:, lo:lo + CH].rearrange("(q p) l -> p q l", p=P),
                    in_=ott.rearrange("p (q l) -> p q l", l=CH))
                prev_out[i] = ott

            # PE warm-up: dummy matmuls on the memset zero tile keep the PE
            # active from right after the initial barrier until the first x
            # chunk lands (~13us), tripping the HAM clock gate to full speed
            # before real work arrives; the result (zeros) lands in an out
            # region that the chunk-0 store overwrites anyway.
            wps = cps_pool.tile([P, CH], F32, tag="cps", name="warmps")
            for _ in range(260):
                nc.tensor.matmul(wps[0:2, 0:2], lhsT=warm_sb, rhs=warm_sb,
                                 start=True, stop=True)
            wout = wpool.tile([2, 2], BF16, tag="warmout")
            nc.scalar.copy(wout, wps[0:2, 0:2])
            nc.gpsimd.dma_start(out=out[2:4, 0:2], in_=wout)

            # chunk pairs, software-pipelined one pair ahead
            emit_conv(0)
            emit_conv(1)
            for p in range(1, NCH // 2):
                emit_conv(2 * p)
                emit_conv(2 * p + 1)
                emit_rest(2 * p - 2)
                emit_rest(2 * p - 1)
            emit_rest(NCH - 2)
            emit_rest(NCH - 1)

            # HAM keep-alive: dummy matmuls chained on the tail's a/v/out
            # tiles keep the Tensor engine active through the drain phase so
            # the activity monitor does not halve the duty cycle under the
            # final scans.
            kps = gps_pool.tile([P, CH], F32, tag="gps", name="keepps")
            for (at, vt) in tail_tiles:
                nc.tensor.matmul(kps[0:2, :], lhsT=warm_sb, rhs=at,
                                 start=True, stop=True)
                nc.tensor.matmul(kps[0:2, :], lhsT=warm_sb, rhs=vt,
                                 start=True, stop=True)
            for ot in range(NOT):
                for i in (NCH - 2, NCH - 1):
                    nc.tensor.matmul(
                        kps[0:2, :], lhsT=warm_sb,
                        rhs=prev_out[i][:, ot * CH:(ot + 1) * CH],
                        start=True, stop=True)
            kout = wpool.tile([2, 2], F32, tag="keepout")
            nc.scalar.copy(kout, kps[0:2, 0:2])
            nc.sync.dma_start(out=dbg[:, :], in_=kout)

    nc.finalize()
    return nc


_PROGRAM = None


def _get_program():
    global _PROGRAM
    if _PROGRAM is None:
        _PROGRAM = build_program()
    return _PROGRAM


def _pmajor(a):
    """[NDT*P, X] -> [P, NDT*X] p-major shuffle: out[p, q*X+x] = a[q*P+p, x]"""
    q = a.shape[0] // P
    return np.ascontiguousarray(
        a.reshape(q, P, -1).transpose(1, 0, 2).reshape(P, -1))


def prepare_in_maps(x, conv_w, h_w, g_w):
    BF = ml_dtypes.bfloat16
    x = np.asarray(x, dtype=np.float32)
    conv_w = np.asarray(conv_w, dtype=np.float32)
    h_w = np.asarray(h_w, dtype=np.float32)
    g_w = np.asarray(g_w, dtype=np.float32)

    xpad = np.zeros((B, D, L + 4), np.float32)
    xpad[:, :, 2:L + 2] = x
    xpad = xpad.astype(BF)                                        # [B,D,L+4]

    hw4d = _pmajor(-h_w[:, :, 0].T).astype(BF)                    # [P, 4*O]
    gw_pad = np.zeros((O, D), np.float32)
    gw_pad[2:, :] = g_w[:, :, 0]
    gw4d = _pmajor(gw_pad.T).astype(BF)                           # [P, 4*O]

    # 5 diagonal blocks per d-tile, p-major [P, 4*5*128]; d-tiles >= NDT_ACT
    # leave the center block unused (that tap rides the DVE merge)
    cwdiag = np.zeros((D, NTAPS * P), np.float32)
    for dt in range(NDT):
        for k in range(NTAPS):
            blk = cwdiag[dt * P:(dt + 1) * P, k * P:(k + 1) * P]
            np.fill_diagonal(blk, conv_w[dt * P:(dt + 1) * P, 0, k])
    cw4d = _pmajor(cwdiag).astype(BF)
    cwm4d = _pmajor(conv_w[:, :, 2]).astype(np.float32)           # [P, 4]

    gbn = np.zeros((O, 1), np.float32)
    gbn[0, 0], gbn[1, 0] = 1000.0, -1000.0     # -(-1000), -(+1000)
    gbn4d = _pmajor(gbn).astype(np.float32)                       # [P, 4]

    zpad = np.zeros((P, 2), BF)
    return [
        {"xp": xpad[b], "hw4d": hw4d, "gw4d": gw4d, "cw4d": cw4d,
         "cwm4d": cwm4d, "gbn4d": gbn4d, "zpad": zpad}
        for b in range(B)
    ]


def kernel(x, conv_w, h_w, g_w):
    in_maps = prepare_in_maps(x, conv_w, h_w, g_w)
    nc = _get_program()
    res = run_bass_kernel_spmd(nc, in_maps, list(range(N_CORES))).results
    return np.stack([np.asarray(res[b]["out"]).astype(np.float32)
                     for b in range(B)], axis=0)


# revision 20
# speedup vs baseline: 1.0139x; 1.0139x over previous
"""MinGRU Trainium2 kernel.

Reference computation (per batch b):
    c = depthwise_conv1d(x, conv_w, taps=5, pad=2)        # [D, L]
    h = h_w @ c                                           # [O, L]
    g = concat([-1000, +1000], g_w @ c)                   # [O, L]
    a = sigmoid(-g); v = (1-a) * h
    out[l] = a[l] * out[l-1] + v[l]     (linear scan along L)

Strategy: pure data-parallel over B (8 batches -> 8 NeuronCores).
Per core, everything streams in l-chunks of 512:
  - all matmul inputs bf16 (x, conv diag weights, h/g weights); PSUM f32
  - x is zero-padded to [D, L+4] on the host, so every chunk (edges
    included) is one single halo-covering DMA
  - conv: taps {0,1,3,4} as diagonal-matmuls on TensorE accumulating in
    PSUM; the center tap is fused into the PSUM->SBUF move as one DVE
    scalar_tensor_tensor (c = x2*w2 + cpsum), which saves a matmul per
    d-tile AND replaces the separate copy pass.  NDT_ACT selects how many
    d-tiles instead keep 5 PE taps with a ScalarE copy (engine rebalance
    knob; 0 = everything on the DVE merge path).
  - weights are uploaded pre-shuffled to p-major [128, ...] layouts so every
    weight DMA is one trigger with large contiguous per-partition rows
  - a = sigmoid(-(g+bias)) on ScalarE; bias carries the +/-1000 polarized
    rows, so scan rows 0/1 equal 0 / h automatically (within error budget)
  - h weights negated on the host so v = (1-a)*h = (a-1)*(-h) is one fused
    DVE scalar_tensor_tensor reading -h straight from PSUM
  - scan via tensor_tensor_scan (DVE), bf16 in/out with f32 internal state
  - dummy matmuls chained on the tail tiles keep the Tensor engine "active"
    through the drain phase so the HAM does not duty-throttle the tail
  - out stored bf16 (one combined DMA per chunk), converted to f32 on host
"""

import numpy as np
import ml_dtypes

import concourse.bass as bass
import concourse.mybir as mybir
from concourse import bacc
from concourse.tile import TileContext
from concourse.bass_utils import run_bass_kernel_spmd

F32 = mybir.dt.float32
BF16 = mybir.dt.bfloat16
AF = mybir.ActivationFunctionType
OP = mybir.AluOpType

B, D, O, L = 8, 512, 512, 4096
P = 128
CH = 512                 # l-chunk width (one PSUM bank)
CHW = CH + 4             # x chunk width incl. 2-col halo each side
NCH = L // CH            # 8
NDT = D // P             # 4 d-tiles
NOT = O // P             # 4 o-tiles
NTAPS = 5
NDT_ACT = 0              # d-tiles whose c-copy runs on ScalarE (5 PE taps)
N_CORES = 8


def build_program():
    nc = bacc.Bacc()

    xp = nc.declare_dram_parameter("xp", [D, L + 4], BF16, isOutput=False)
    # p-major pre-shuffled weight layouts (see prepare_in_maps)
    hw4d = nc.declare_dram_parameter("hw4d", [P, NDT * O], BF16, isOutput=False)
    gw4d = nc.declare_dram_parameter("gw4d", [P, NDT * O], BF16, isOutput=False)
    cw4d = nc.declare_dram_parameter("cw4d", [P, NDT * NTAPS * P], BF16,
                                     isOutput=False)
    cwm4d = nc.declare_dram_parameter("cwm4d", [P, NDT], F32, isOutput=False)
    gbn4d = nc.declare_dram_parameter("gbn4d", [P, NOT], F32, isOutput=False)
    zpad = nc.declare_dram_parameter("zpad", [P, 2], BF16, isOutput=False)
    out = nc.declare_dram_parameter("out", [O, L], BF16, isOutput=True)
    dbg = nc.declare_dram_parameter("dbg", [2, 2], F32, isOutput=True)

    CWB = NTAPS * P          # cw block stride per d-tile

    with TileContext(nc) as tc:
        with (
            tc.tile_pool(name="weights", bufs=1) as wpool,
            tc.tile_pool(name="xin", bufs=4) as xpool,
            tc.tile_pool(name="csb", bufs=10) as cpool,
            tc.tile_pool(name="actout", bufs=12) as apool,
            tc.tile_pool(name="vtiles", bufs=12) as vpool,
            tc.tile_pool(name="outt", bufs=3) as opool,
            tc.tile_pool(name="cps", bufs=3, space="PSUM") as cps_pool,
            tc.tile_pool(name="hps", bufs=3, space="PSUM") as hps_pool,
            tc.tile_pool(name="gps", bufs=2, space="PSUM") as gps_pool,
        ):
            # The warm-up tile comes from a GpSimd memset (no DMA), so the PE
            # can start its HAM-tripping dummy matmuls as soon as the initial
            # barrier clears instead of waiting ~10us for the DMA rings.
            # Scalar queue: conv weights (d-tiles 0,1 first so chunk-0 conv
            # starts early), then x chunks 2..7. Sync queue: x chunks 0,1
            # then the stores. GpSimd SWDGE: the two big h/g matrices.
            warm_sb = wpool.tile([P, 2], BF16, tag="warm")
            nc.gpsimd.memset(warm_sb, 0.0)
            warm2 = wpool.tile([P, CH], BF16, tag="warm2")
            nc.gpsimd.memset(warm2, 0.0)
            # per-d-tile conv weight loads: the dt-0 block lands first so
            # chunk-0 conv starts as early as possible
            cw_sb = []
            for dt in range(NDT):
                t = wpool.tile([P, CWB], BF16, tag=f"cw{dt}")
                nc.scalar.dma_start(out=t,
                                    in_=cw4d[:, dt * CWB:(dt + 1) * CWB])
                cw_sb.append(t)
            cwm4 = wpool.tile([P, NDT], F32, tag="cwm4")
            nc.scalar.dma_start(out=cwm4, in_=cwm4d[:, :])
            gbn4 = wpool.tile([P, NOT], F32, tag="gbn4")
            nc.scalar.dma_start(out=gbn4, in_=gbn4d[:, :])
            gw4 = wpool.tile([P, NDT * O], BF16, tag="gw4")
            nc.gpsimd.dma_start(out=gw4, in_=gw4d[:, :])
            hw4 = wpool.tile([P, NDT * O], BF16, tag="hw4")
            nc.gpsimd.dma_start(out=hw4, in_=hw4d[:, :])

            def cwblk(dt, k):
                return cw_sb[dt][:, k * P:(k + 1) * P]

            c_sb = [None] * NCH          # [chunk] -> list of 4 SBUF c tiles
            prev_out = [None] * NCH      # [chunk] -> combined out tile
            tail_tiles = []              # late tiles for the HAM keep-alive

            def emit_conv(i):
                lo = i * CH
                # one combined x DMA for all 4 d-tiles (halo included; x is
                # pre-padded on the host so edge chunks need no special case).
                # The first two chunks load the d-tile-0 slice as its own
                # tile+DMA so chunk-0 conv can start as soon as it lands.
                if i < 2:
                    xta = xpool.tile([P, CHW], BF16, tag="xta")
                    nc.sync.dma_start(
                        out=xta,
                        in_=xp[0:P, lo:lo + CHW])
                    xt = xpool.tile([P, (NDT - 1) * CHW], BF16, tag="xtb")
                    nc.sync.dma_start(
                        out=xt.rearrange("p (q c) -> p q c", c=CHW),
                        in_=xp[P:, lo:lo + CHW].rearrange(
                            "(q p) l -> p q l", p=P))

                    def xs(dt, k):
                        if dt == 0:
                            return xta[:, k:k + CH]
                        return xt[:, (dt - 1) * CHW + k:(dt - 1) * CHW + k + CH]
                else:
                    xt = xpool.tile([P, NDT * CHW], BF16, tag="xt")
                    nc.scalar.dma_start(
                        out=xt.rearrange("p (q c) -> p q c", c=CHW),
                        in_=xp[:, lo:lo + CHW].rearrange(
                            "(q p) l -> p q l", p=P))

                    def xs(dt, k):
                        return xt[:, dt * CHW + k:dt * CHW + k + CH]
                tiles = []
                for dt in range(NDT):
                    on_act = dt < NDT_ACT
                    cp = cps_pool.tile([P, CH], F32, tag="cps")
                    taps = (1, 3, 0, 2, 4) if on_act else (1, 3, 0, 4)
                    for j, k in enumerate(taps):
                        nc.tensor.matmul(
                            cp,
                            lhsT=cwblk(dt, k),
                            rhs=xs(dt, k),
                            start=(j == 0), stop=(j == len(taps) - 1),
                        )
                    ct = cpool.tile([P, CH], BF16, tag="ct")
                    if on_act:
                        nc.scalar.copy(ct, cp)
                    else:
                        # center tap fused with the PSUM->SBUF move
                        nc.vector.scalar_tensor_tensor(
                            ct, xs(dt, 2), cwm4[:, dt:dt + 1], cp,
                            OP.mult, OP.add)
                    tiles.append(ct)
                c_sb[i] = tiles

            def emit_rest(i):
                lo = i * CH
                ott = opool.tile([P, NOT * CH], BF16, tag="ott")
                for ot in range(NOT):
                    # g before h: the sigmoid chain (ACT) only needs g, so it
                    # starts while the h matmuls are still streaming
                    gp = gps_pool.tile([P, CH], F32, tag="gps")
                    for dt in range(NDT):
                        nc.tensor.matmul(
                            gp,
                            lhsT=gw4[:, dt * O + ot * P:dt * O + (ot + 1) * P],
                            rhs=c_sb[i][dt],
                            start=(dt == 0), stop=(dt == NDT - 1),
                        )
                    hp = hps_pool.tile([P, CH], F32, tag="hps")
                    for dt in range(NDT):
                        nc.tensor.matmul(
                            hp,
                            lhsT=hw4[:, dt * O + ot * P:dt * O + (ot + 1) * P],
                            rhs=c_sb[i][dt],
                            start=(dt == 0), stop=(dt == NDT - 1),
                        )
                    # a = sigmoid(-(g + bias)); v = (a-1)*(-h)
                    at = apool.tile([P, CH], BF16, tag="at")
                    nc.scalar.activation(at, gp, AF.Sigmoid,
                                         bias=gbn4[:, ot:ot + 1], scale=-1.0)
                    vt = vpool.tile([P, CH], BF16, tag="vt")
                    nc.vector.scalar_tensor_tensor(vt, at, 1.0, hp,
                                                   OP.subtract, OP.mult)
                    init = (0.0 if i == 0 else
                            prev_out[i - 1][:, ot * CH + CH - 1:ot * CH + CH])
                    nc.vector.tensor_tensor_scan(
                        ott[:, ot * CH:(ot + 1) * CH], at, vt, init,
                        OP.mult, OP.add)
                    if i >= NCH - 2:
                        tail_tiles.append((at, vt))
                nc.sync.dma_start(
                    out=out[:, lo:lo + CH].rearrange("(q p) l -> p q l", p=P),
                    in_=ott.rearrange("p (q l) -> p q l", l=CH))
                prev_out[i] = ott

            # PE warm-up: dummy matmuls on the memset zero tiles keep the PE
            # active from right after the initial barrier until the first x
            # chunk lands (~13us). The tiny [2,2] ones start instantly; the
            # full-width [2,512] ones stream real column activity so the HAM
            # sees genuine utilization and lifts the duty gate before real
            # work arrives. Results (zeros) land in an out region that the
            # chunk-0 store overwrites anyway.
            wps = cps_pool.tile([P, CH], F32, tag="cps", name="warmps")
            for _ in range(60):
                nc.tensor.matmul(wps[0:2, 0:2], lhsT=warm_sb, rhs=warm_sb,
                                 start=True, stop=True)
            for _ in range(12):
                nc.tensor.matmul(wps[0:2, :], lhsT=warm_sb, rhs=warm2,
                                 start=True, stop=True)
            wout = wpool.tile([2, 2], BF16, tag="warmout")
            nc.scalar.copy(wout, wps[0:2, 0:2])
            nc.gpsimd.dma_start(out=out[2:4, 0:2], in_=wout)

            # software pipeline, alternating conv/rest with a two-chunk
            # lookahead: between conv(i+2) and conv(i+3) on the PE queue sits
            # a full rest(i) (~7us of matmuls), so the DVE merge-stt that
            # frees conv's PSUM slots is never on the PE's critical path
            emit_conv(0)
            emit_conv(1)
            for i in range(NCH):
                if i + 2 < NCH:
                    emit_conv(i + 2)
                emit_rest(i)

            # HAM keep-alive: dummy matmuls chained on the tail's a/v/out
            # tiles keep the Tensor engine active through the drain phase so
            # the activity monitor does not halve the duty cycle under the
            # final scans.
            kps = gps_pool.tile([P, CH], F32, tag="gps", name="keepps")
            for (at, vt) in tail_tiles:
                nc.tensor.matmul(kps[0:2, :], lhsT=warm_sb, rhs=at,
                                 start=True, stop=True)
                nc.tensor.matmul(kps[0:2, :], lhsT=warm_sb, rhs=vt,
                                 start=True, stop=True)
            for ot in range(NOT):
                for i in (NCH - 2, NCH - 1):
                    nc.tensor.matmul(
                        kps[0:2, :], lhsT=warm_sb,
                        rhs=prev_out[i][:, ot * CH:(ot + 1) * CH],
                        start=True, stop=True)
            kout = wpool.tile([2, 2], F32, tag="keepout")
            nc.scalar.copy(kout, kps[0:2, 0:2])
            nc.sync.dma_start(out=dbg[:, :], in_=kout)

    nc.finalize()
    return nc


_PROGRAM = None


def _get_program():
    global _PROGRAM
    if _PROGRAM is None:
        _PROGRAM = build_program()
    return _PROGRAM


def _pmajor(a):
    """[NDT*P, X] -> [P, NDT*X] p-major shuffle: out[p, q*X+x] = a[q*P+p, x]"""
    q = a.shape[0] // P
    return np.ascontiguousarray(
        a.reshape(q, P, -1).transpose(1, 0, 2).reshape(P, -1))


def prepare_in_maps(x, conv_w, h_w, g_w):
    BF = ml_dtypes.bfloat16
    x = np.asarray(x, dtype=np.float32)
    conv_w = np.asarray(conv_w, dtype=np.float32)
    h_w = np.asarray(h_w, dtype=np.float32)
    g_w = np.asarray(g_w, dtype=np.float32)

    xpad = np.zeros((B, D, L + 4), np.float32)
    xpad[:, :, 2:L + 2] = x
    xpad = xpad.astype(BF)                                        # [B,D,L+4]

    hw4d = _pmajor(-h_w[:, :, 0].T).astype(BF)                    # [P, 4*O]
    gw_pad = np.zeros((O, D), np.float32)
    gw_pad[2:, :] = g_w[:, :, 0]
    gw4d = _pmajor(gw_pad.T).astype(BF)                           # [P, 4*O]

    # 5 diagonal blocks per d-tile, p-major [P, 4*5*128]; d-tiles >= NDT_ACT
    # leave the center block unused (that tap rides the DVE merge)
    cwdiag = np.zeros((D, NTAPS * P), np.float32)
    for dt in range(NDT):
        for k in range(NTAPS):
            blk = cwdiag[dt * P:(dt + 1) * P, k * P:(k + 1) * P]
            np.fill_diagonal(blk, conv_w[dt * P:(dt + 1) * P, 0, k])
    cw4d = _pmajor(cwdiag).astype(BF)
    cwm4d = _pmajor(conv_w[:, :, 2]).astype(np.float32)           # [P, 4]

    gbn = np.zeros((O, 1), np.float32)
    gbn[0, 0], gbn[1, 0] = 1000.0, -1000.0     # -(-1000), -(+1000)
    gbn4d = _pmajor(gbn).astype(np.float32)                       # [P, 4]

    zpad = np.zeros((P, 2), BF)
    return [
        {"xp": xpad[b], "hw4d": hw4d, "gw4d": gw4d, "cw4d": cw4d,
         "cwm4d": cwm4d, "gbn4d": gbn4d, "zpad": zpad}
        for b in range(B)
    ]


def kernel(x, conv_w, h_w, g_w):
    in_maps = prepare_in_maps(x, conv_w, h_w, g_w)
    nc = _get_program()
    res = run_bass_kernel_spmd(nc, in_maps, list(range(N_CORES))).results
    return np.stack([np.asarray(res[b]["out"]).astype(np.float32)
                     for b in range(B)], axis=0)


# revision 22
# speedup vs baseline: 1.0282x; 1.0141x over previous
"""MinGRU Trainium2 kernel.

Reference computation (per batch b):
    c = depthwise_conv1d(x, conv_w, taps=5, pad=2)        # [D, L]
    h = h_w @ c                                           # [O, L]
    g = concat([-1000, +1000], g_w @ c)                   # [O, L]
    a = sigmoid(-g); v = (1-a) * h
    out[l] = a[l] * out[l-1] + v[l]     (linear scan along L)

Strategy: pure data-parallel over B (8 batches -> 8 NeuronCores).
Per core, everything streams in l-chunks of 512:
  - all matmul inputs bf16 (x, conv diag weights, h/g weights); PSUM f32
  - x is zero-padded to [D, L+4] on the host, so every chunk (edges
    included) is one single halo-covering DMA
  - conv: taps {0,1,3,4} as diagonal-matmuls on TensorE accumulating in
    PSUM; the center tap is fused into the PSUM->SBUF move as one DVE
    scalar_tensor_tensor (c = x2*w2 + cpsum), which saves a matmul per
    d-tile AND replaces the separate copy pass.  NDT_ACT selects how many
    d-tiles instead keep 5 PE taps with a ScalarE copy (engine rebalance
    knob; 0 = everything on the DVE merge path).
  - weights are uploaded pre-shuffled to p-major [128, ...] layouts so every
    weight DMA is one trigger with large contiguous per-partition rows
  - a = sigmoid(-(g+bias)) on ScalarE; bias carries the +/-1000 polarized
    rows, so scan rows 0/1 equal 0 / h automatically (within error budget)
  - h weights negated on the host so v = (1-a)*h = (a-1)*(-h) is one fused
    DVE scalar_tensor_tensor reading -h straight from PSUM
  - scan via tensor_tensor_scan (DVE), bf16 in/out with f32 internal state
  - dummy matmuls chained on the tail tiles keep the Tensor engine "active"
    through the drain phase so the HAM does not duty-throttle the tail
  - out stored bf16 (one combined DMA per chunk), converted to f32 on host
"""

import numpy as np
import ml_dtypes

import concourse.bass as bass
import concourse.mybir as mybir
from concourse import bacc
from concourse.tile import TileContext
from concourse.bass_utils import run_bass_kernel_spmd

F32 = mybir.dt.float32
BF16 = mybir.dt.bfloat16
AF = mybir.ActivationFunctionType
OP = mybir.AluOpType

B, D, O, L = 8, 512, 512, 4096
P = 128
CH = 512                 # l-chunk width (one PSUM bank)
CHW = CH + 4             # x chunk width incl. 2-col halo each side
NCH = L // CH            # 8
NDT = D // P             # 4 d-tiles
NOT = O // P             # 4 o-tiles
NTAPS = 5
NDT_ACT = 0              # d-tiles whose c-copy runs on ScalarE (5 PE taps)
N_CORES = 8


def build_program():
    nc = bacc.Bacc()

    xp = nc.declare_dram_parameter("xp", [D, L + 4], BF16, isOutput=False)
    # p-major pre-shuffled weight layouts (see prepare_in_maps)
    hw4d = nc.declare_dram_parameter("hw4d", [P, NDT * O], BF16, isOutput=False)
    gw4d = nc.declare_dram_parameter("gw4d", [P, NDT * O], BF16, isOutput=False)
    cw4d = nc.declare_dram_parameter("cw4d", [P, NDT * NTAPS * P], BF16,
                                     isOutput=False)
    cwm4d = nc.declare_dram_parameter("cwm4d", [P, NDT], F32, isOutput=False)
    gbn4d = nc.declare_dram_parameter("gbn4d", [P, NOT], F32, isOutput=False)
    zpad = nc.declare_dram_parameter("zpad", [P, 2], BF16, isOutput=False)
    out = nc.declare_dram_parameter("out", [O, L], BF16, isOutput=True)
    dbg = nc.declare_dram_parameter("dbg", [2, 2], F32, isOutput=True)

    CWB = NTAPS * P          # cw block stride per d-tile

    with TileContext(nc) as tc:
        with (
            tc.tile_pool(name="weights", bufs=1) as wpool,
            tc.tile_pool(name="xin", bufs=4) as xpool,
            tc.tile_pool(name="csb", bufs=10) as cpool,
            tc.tile_pool(name="actout", bufs=12) as apool,
            tc.tile_pool(name="vtiles", bufs=12) as vpool,
            tc.tile_pool(name="outt", bufs=3) as opool,
            tc.tile_pool(name="cps", bufs=3, space="PSUM") as cps_pool,
            tc.tile_pool(name="hps", bufs=3, space="PSUM") as hps_pool,
            tc.tile_pool(name="gps", bufs=2, space="PSUM") as gps_pool,
        ):
            # The warm-up tile comes from a GpSimd memset (no DMA), so the PE
            # can start its HAM-tripping dummy matmuls as soon as the initial
            # barrier clears instead of waiting ~10us for the DMA rings.
            # Scalar queue: conv weights (d-tiles 0,1 first so chunk-0 conv
            # starts early), then x chunks 2..7. Sync queue: x chunks 0,1
            # then the stores. GpSimd SWDGE: the two big h/g matrices.
            warm_sb = wpool.tile([P, 2], BF16, tag="warm")
            nc.gpsimd.memset(warm_sb, 0.0)
            warm2 = wpool.tile([P, CH], BF16, tag="warm2")
            nc.gpsimd.memset(warm2, 0.0)
            # per-d-tile conv weight loads: the dt-0 block lands first so
            # chunk-0 conv starts as early as possible
            cw_sb = []
            for dt in range(NDT):
                t = wpool.tile([P, CWB], BF16, tag=f"cw{dt}")
                nc.scalar.dma_start(out=t,
                                    in_=cw4d[:, dt * CWB:(dt + 1) * CWB])
                cw_sb.append(t)
            cwm4 = wpool.tile([P, NDT], F32, tag="cwm4")
            nc.scalar.dma_start(out=cwm4, in_=cwm4d[:, :])
            gbn4 = wpool.tile([P, NOT], F32, tag="gbn4")
            nc.scalar.dma_start(out=gbn4, in_=gbn4d[:, :])
            gw4 = wpool.tile([P, NDT * O], BF16, tag="gw4")
            nc.gpsimd.dma_start(out=gw4, in_=gw4d[:, :])
            hw4 = wpool.tile([P, NDT * O], BF16, tag="hw4")
            nc.gpsimd.dma_start(out=hw4, in_=hw4d[:, :])

            def cwblk(dt, k):
                return cw_sb[dt][:, k * P:(k + 1) * P]

            c_sb = [None] * NCH          # [chunk] -> list of 4 SBUF c tiles
            prev_out = [None] * NCH      # [chunk] -> combined out tile
            tail_tiles = []              # late tiles for the HAM keep-alive

            def emit_conv(i):
                lo = i * CH
                # one combined x DMA for all 4 d-tiles (halo included; x is
                # pre-padded on the host so edge chunks need no special case).
                # The first two chunks load the d-tile-0 slice as its own
                # tile+DMA so chunk-0 conv can start as soon as it lands.
                if i < 2:
                    xta = xpool.tile([P, CHW], BF16, tag="xta")
                    nc.sync.dma_start(
                        out=xta,
                        in_=xp[0:P, lo:lo + CHW])
                    xt = xpool.tile([P, (NDT - 1) * CHW], BF16, tag="xtb")
                    nc.sync.dma_start(
                        out=xt.rearrange("p (q c) -> p q c", c=CHW),
                        in_=xp[P:, lo:lo + CHW].rearrange(
                            "(q p) l -> p q l", p=P))

                    def xs(dt, k):
                        if dt == 0:
                            return xta[:, k:k + CH]
                        return xt[:, (dt - 1) * CHW + k:(dt - 1) * CHW + k + CH]
                else:
                    xt = xpool.tile([P, NDT * CHW], BF16, tag="xt")
                    nc.scalar.dma_start(
                        out=xt.rearrange("p (q c) -> p q c", c=CHW),
                        in_=xp[:, lo:lo + CHW].rearrange(
                            "(q p) l -> p q l", p=P))

                    def xs(dt, k):
                        return xt[:, dt * CHW + k:dt * CHW + k + CH]
                tiles = []
                for dt in range(NDT):
                    on_act = dt < NDT_ACT
                    cp = cps_pool.tile([P, CH], F32, tag="cps")
                    taps = (1, 3, 0, 2, 4) if on_act else (1, 3, 0, 4)
                    for j, k in enumerate(taps):
                        nc.tensor.matmul(
                            cp,
                            lhsT=cwblk(dt, k),
                            rhs=xs(dt, k),
                            start=(j == 0), stop=(j == len(taps) - 1),
                        )
                    ct = cpool.tile([P, CH], BF16, tag="ct")
                    if on_act:
                        nc.scalar.copy(ct, cp)
                    else:
                        # center tap fused with the PSUM->SBUF move
                        nc.vector.scalar_tensor_tensor(
                            ct, xs(dt, 2), cwm4[:, dt:dt + 1], cp,
                            OP.mult, OP.add)
                    tiles.append(ct)
                c_sb[i] = tiles

            def emit_rest(i):
                lo = i * CH
                ott = opool.tile([P, NOT * CH], BF16, tag="ott")
                for ot in range(NOT):
                    # g before h: the sigmoid chain (ACT) only needs g, so it
                    # starts while the h matmuls are still streaming
                    gp = gps_pool.tile([P, CH], F32, tag="gps")
                    for dt in range(NDT):
                        nc.tensor.matmul(
                            gp,
                            lhsT=gw4[:, dt * O + ot * P:dt * O + (ot + 1) * P],
                            rhs=c_sb[i][dt],
                            start=(dt == 0), stop=(dt == NDT - 1),
                        )
                    hp = hps_pool.tile([P, CH], F32, tag="hps")
                    for dt in range(NDT):
                        nc.tensor.matmul(
                            hp,
                            lhsT=hw4[:, dt * O + ot * P:dt * O + (ot + 1) * P],
                            rhs=c_sb[i][dt],
                            start=(dt == 0), stop=(dt == NDT - 1),
                        )
                    # a = sigmoid(-(g + bias)); v = (a-1)*(-h)
                    at = apool.tile([P, CH], BF16, tag="at")
                    nc.scalar.activation(at, gp, AF.Sigmoid,
                                         bias=gbn4[:, ot:ot + 1], scale=-1.0)
                    vt = vpool.tile([P, CH], BF16, tag="vt")
                    nc.vector.scalar_tensor_tensor(vt, at, 1.0, hp,
                                                   OP.subtract, OP.mult)
                    init = (0.0 if i == 0 else
                            prev_out[i - 1][:, ot * CH + CH - 1:ot * CH + CH])
                    nc.vector.tensor_tensor_scan(
                        ott[:, ot * CH:(ot + 1) * CH], at, vt, init,
                        OP.mult, OP.add)
                    if i >= NCH - 2:
                        tail_tiles.append((at, vt))
                    if i == NCH - 1:
                        # last chunk: store each o-tile as soon as its scan
                        # lands instead of waiting for the whole chunk
                        nc.sync.dma_start(
                            out=out[ot * P:(ot + 1) * P, lo:lo + CH],
                            in_=ott[:, ot * CH:(ot + 1) * CH])
                if i < NCH - 1:
                    nc.sync.dma_start(
                        out=out[:, lo:lo + CH].rearrange(
                            "(q p) l -> p q l", p=P),
                        in_=ott.rearrange("p (q l) -> p q l", l=CH))
                prev_out[i] = ott

            # PE warm-up: dummy matmuls on the memset zero tiles keep the PE
            # active from right after the initial barrier until the first x
            # chunk lands (~13us). The tiny [2,2] ones start instantly; the
            # full-width [2,512] ones stream real column activity so the HAM
            # sees genuine utilization and lifts the duty gate before real
            # work arrives. Results (zeros) land in an out region that the
            # chunk-0 store overwrites anyway.
            wps = cps_pool.tile([P, CH], F32, tag="cps", name="warmps")
            for _ in range(60):
                nc.tensor.matmul(wps[0:2, 0:2], lhsT=warm_sb, rhs=warm_sb,
                                 start=True, stop=True)
            # enough full-width warmups to bridge seamlessly into the first
            # conv matmuls (~17us): any activity dip re-triggers a ~7us
            # half-duty window right as real work starts
            for _ in range(28):
                nc.tensor.matmul(wps[0:2, :], lhsT=warm_sb, rhs=warm2,
                                 start=True, stop=True)
            wout = wpool.tile([2, 2], BF16, tag="warmout")
            nc.scalar.copy(wout, wps[0:2, 0:2])
            nc.gpsimd.dma_start(out=out[2:4, 0:2], in_=wout)

            # software pipeline, alternating conv/rest with a two-chunk
            # lookahead: between conv(i+2) and conv(i+3) on the PE queue sits
            # a full rest(i) (~7us of matmuls), so the DVE merge-stt that
            # frees conv's PSUM slots is never on the PE's critical path
            emit_conv(0)
            emit_conv(1)
            for i in range(NCH):
                if i + 2 < NCH:
                    emit_conv(i + 2)
                emit_rest(i)

            # HAM keep-alive: dummy matmuls chained on the tail's a/v/out
            # tiles keep the Tensor engine active through the drain phase so
            # the activity monitor does not halve the duty cycle under the
            # final scans.
            kps = gps_pool.tile([P, CH], F32, tag="gps", name="keepps")
            for (at, vt) in tail_tiles:
                nc.tensor.matmul(kps[0:2, :], lhsT=warm_sb, rhs=at,
                                 start=True, stop=True)
                nc.tensor.matmul(kps[0:2, :], lhsT=warm_sb, rhs=vt,
                                 start=True, stop=True)
            for ot in range(NOT):
                for i in (NCH - 2, NCH - 1):
                    nc.tensor.matmul(
                        kps[0:2, :], lhsT=warm_sb,
                        rhs=prev_out[i][:, ot * CH:(ot + 1) * CH],
                        start=True, stop=True)
            kout = wpool.tile([2, 2], F32, tag="keepout")
            nc.scalar.copy(kout, kps[0:2, 0:2])
            nc.sync.dma_start(out=dbg[:, :], in_=kout)

    nc.finalize()
    return nc


_PROGRAM = None


def _get_program():
    global _PROGRAM
    if _PROGRAM is None:
        _PROGRAM = build_program()
    return _PROGRAM


def _pmajor(a):
    """[NDT*P, X] -> [P, NDT*X] p-major shuffle: out[p, q*X+x] = a[q*P+p, x]"""
    q = a.shape[0] // P
    return np.ascontiguousarray(
        a.reshape(q, P, -1).transpose(1, 0, 2).reshape(P, -1))


def prepare_in_maps(x, conv_w, h_w, g_w):
    BF = ml_dtypes.bfloat16
    x = np.asarray(x, dtype=np.float32)
    conv_w = np.asarray(conv_w, dtype=np.float32)
    h_w = np.asarray(h_w, dtype=np.float32)
    g_w = np.asarray(g_w, dtype=np.float32)

    xpad = np.zeros((B, D, L + 4), np.float32)
    xpad[:, :, 2:L + 2] = x
    xpad = xpad.astype(BF)                                        # [B,D,L+4]

    hw4d = _pmajor(-h_w[:, :, 0].T).astype(BF)                    # [P, 4*O]
    gw_pad = np.zeros((O, D), np.float32)
    gw_pad[2:, :] = g_w[:, :, 0]
    gw4d = _pmajor(gw_pad.T).astype(BF)                           # [P, 4*O]

    # 5 diagonal blocks per d-tile, p-major [P, 4*5*128]; d-tiles >= NDT_ACT
    # leave the center block unused (that tap rides the DVE merge)
    cwdiag = np.zeros((D, NTAPS * P), np.float32)
    for dt in range(NDT):
        for k in range(NTAPS):
            blk = cwdiag[dt * P:(dt + 1) * P, k * P:(k + 1) * P]
            np.fill_diagonal(blk, conv_w[dt * P:(dt + 1) * P, 0, k])
    cw4d = _pmajor(cwdiag).astype(BF)
    cwm4d = _pmajor(conv_w[:, :, 2]).astype(np.float32)           # [P, 4]

    gbn = np.zeros((O, 1), np.float32)
    gbn[0, 0], gbn[1, 0] = 1000.0, -1000.0     # -(-1000), -(+1000)
    gbn4d = _pmajor(gbn).astype(np.float32)                       # [P, 4]

    zpad = np.zeros((P, 2), BF)
    return [
        {"xp": xpad[b], "hw4d": hw4d, "gw4d": gw4d, "cw4d": cw4d,
         "cwm4d": cwm4d, "gbn4d": gbn4d, "zpad": zpad}
        for b in range(B)
    ]


def kernel(x, conv_w, h_w, g_w):
    in_maps = prepare_in_maps(x, conv_w, h_w, g_w)
    nc = _get_program()
    res = run_bass_kernel_spmd(nc, in_maps, list(range(N_CORES))).results
    return np.stack([np.asarray(res[b]["out"]).astype(np.float32)
                     for b in range(B)], axis=0)
